# revision 14
# baseline (speedup 1.0000x reference)
"""NPairLoss on 8 TRN2 NeuronCores.

loss = lw/n * sum_i log(sum_j exp(cos(w_i, w_j) - 1))   for W [256, 16384]

Strategy (SPMD, no collectives):
  - Host: column-normalize W in float64, downcast to bf16 (wn). Core k owns
    Gram rows k*2048..(k+1)*2048-1; it receives wn rolled by -k*2048 columns
    so every core's program is identical (its rows are local cols 0..2047).
  - Device per core: Gram block row-tiles via PE into PSUM ([128,2048] f32
    spans, double buffered), exp(G-1) row-sums via one ACT pass per span
    using accum_out. ACT-bound at ~250us/core; PE ~220us hidden under it.
  - Output per core: [128, 128] f32 partial row sums (16 m-tiles x 8 groups).
  - Host: S = sum over groups, loss = lw * sum(log(S)) / n in float64.
"""

import numpy as np

import bass_rust
import concourse.bass as bass
import concourse.tile as tile
from concourse import mybir
from concourse._compat import with_exitstack
from concourse.bass_utils import run_bass_kernel_spmd

D = 256
N = 16384
NCORES = 8
RB = N // NCORES          # 2048 rows per core
GRP = 2048                # rhs group width == one PSUM tile span (4 banks)
CH = 512                  # matmul moving free dim
MT = RB // 128            # 16 row tiles per core
NG = N // GRP             # 8 rhs groups
NC_CH = GRP // CH         # 4 chunks per group

F32 = mybir.dt.float32
BF16 = mybir.dt.bfloat16
AF = mybir.ActivationFunctionType

TRACE = False
LAST_EXEC_NS = None
LAST_IN_MAPS = None


@with_exitstack
def _npair_tile_kernel(ctx, tc, out_ap, wn_ap, reps=1):
    nc = tc.nc

    epool = ctx.enter_context(tc.tile_pool(name="expout", bufs=2))
    psum = ctx.enter_context(
        tc.tile_pool(name="psum", bufs=2, space=bass.MemorySpace.PSUM)
    )
    singles = ctx.enter_context(tc.tile_pool(name="singles", bufs=1))

    neg1 = singles.tile([128, 1], F32)
    nc.any.memset(neg1, -1.0)
    # wn[h]: bf16 column-normalized W, K-half h on partitions.
    wn = [singles.tile([128, N], BF16, name=f"wn{h}") for h in range(2)]
    # accs[:, m*NG+g] = sum_j-in-group-g exp(G[m*128+p, j] - 1)
    accs = singles.tile([128, MT * NG], F32)

    for g in range(NG):
        for h in range(2):
            eng = nc.sync if h == 0 else nc.gpsimd
            eng.dma_start(
                wn[h][:, g * GRP:(g + 1) * GRP],
                wn_ap[h * 128:(h + 1) * 128, g * GRP:(g + 1) * GRP],
            )

    def body(pipe=None, iv=None):
        for g in range(NG):
            # Absorb this group's input-DMA waits so the first matmul stays
            # within walrus's per-instruction sync-wait budget.
            for h in range(2):
                nc.tensor.ldweights(wn[h][:, g * GRP:g * GRP + 128])
            for m in range(MT):
                ps = psum.tile([128, GRP], F32, name="ps")
                for h in range(2):
                    for c in range(NC_CH):
                        nc.tensor.matmul(
                            ps[:, c * CH:(c + 1) * CH],
                            wn[h][:, m * 128:(m + 1) * 128],
                            wn[h][:, g * GRP + c * CH:g * GRP + (c + 1) * CH],
                            start=(h == 0),
                            stop=(h == 1),
                        )
                eo = epool.tile([128, GRP], BF16, name="eo")
                col = m * NG + g
                nc.scalar.activation(
                    eo[:], ps[:], AF.Exp, bias=neg1[:],
                    accum_out=accs[:, col:col + 1],
                )

    if reps == 1:
        body()
    else:
        tc.For_i_pipelined([body], 0, reps)

    nc.sync.dma_start(out_ap[:], accs[:])


def _build_program(reps=1):
    nc = bass.Bass("TRN2", target_bir_lowering=False, debug=False,
                   num_devices=NCORES)
    wn = nc.dram_tensor("wn", [D, N], BF16, kind="ExternalInput").ap()
    out = nc.dram_tensor("out", [128, MT * NG], F32, kind="ExternalOutput").ap()
    with tile.TileContext(nc) as tc:
        _npair_tile_kernel(tc, out, wn, reps=reps)
    # Walrus enforces per-instruction sync-wait slot limits (ACT allows just
    # one); split multi-waits into event semaphores like Bacc.compile does.
    bass_rust.move_matmul_waits_to_ldweights(nc.m)
    bass_rust.generate_event_semaphores(nc)
    return nc


_NC_CACHE = None


def kernel(**inputs) -> np.ndarray:
    global _NC_CACHE, LAST_EXEC_NS, LAST_IN_MAPS
    w = np.asarray(inputs["weight"], dtype=np.float32)
    lw = np.float64(np.asarray(inputs["loss_weight"]))
    assert w.shape == (D, N)

    wd = w.astype(np.float64)
    norms = np.sqrt((wd * wd).sum(axis=0))
    wn = wd / np.maximum(norms, 1e-8)
    wn16 = wn.astype(mybir.dt.np(BF16))

    if _NC_CACHE is None:
        _NC_CACHE = _build_program()
    nc = _NC_CACHE

    in_maps = [
        {"wn": np.ascontiguousarray(np.roll(wn16, -k * RB, axis=1))}
        for k in range(NCORES)
    ]
    LAST_IN_MAPS = in_maps
    res = run_bass_kernel_spmd(nc, in_maps, list(range(NCORES)), trace=TRACE)
    LAST_EXEC_NS = res.exec_time_ns

    parts = np.stack([np.asarray(res.results[k]["out"]) for k in range(NCORES)])
    # parts: [8, 128, 128]; col = m*NG + g
    S = parts.astype(np.float64).reshape(NCORES, 128, MT, NG).sum(axis=3)
    loss = lw * np.log(S).sum() / N
    return np.asarray(loss, dtype=np.float32)
